# revision 1
# baseline (speedup 1.0000x reference)
"""CVQNN classifier kernel for 8 Trainium2 NeuronCores.

Math: the whole quantum circuit collapses to a batch-independent affine map
(S, d) on 128-dim phase space.  Per batch row the heavy work is
    msel' = x @ W2 + d20/2          (W2 = S[rows, :64].T, shape (64, 20))
    out_k = log1p(relu(msel'_x[k]^2 + msel'_p[k]^2 + cov_k/4 - 0.5))
i.e. a (B,64) @ (64,20) matmul + elementwise tail -> (B,10).  Memory bound.

Device layout (per core, R = 125952 rows):
  - host splits x into bf16 hi/lo (x = xh + xl exactly to ~2^-17 rel) and
    packs xstack (128, R) bf16: partitions 0..63 = xh features, 64..127 =
    xl features.  Same DMA bytes as fp32 x, but the PE runs single-pass
    bf16 with FWL weight loads instead of double-pass fp32.
  - per super-block (6144 cols = 48 j-blocks): 1 DMA [128, 6144] bf16
    (12 KB per-partition descriptors).  One matmul per j-block,
    stationary = xstack_j [128, 128], moving = wcat [128, 40] =
    [[Wh, Wl], [Wh, 0]]:
      psum cols 0..19  = xh.Wh + xl.Wh   (K-sum does the hi+lo merge)
      psum cols 20..39 = xh.Wl           (correction, merged on DVE)
    (dropped xl.Wl term ~ 2^-18).  One 4-bank psum tile per super-block,
    12 j-blocks in the first 480 cols of each 512-col bank.
  - tail: t2 = r1 + (r2 + d) on DVE (folds the d-add into the hi/lo
    merge; never two PSUM operands in one op), then square (ACT),
    pair-add + cov-add (DVE), relu + ln(1+.) (ACT).
  - DMA out [128, 480]: per-partition 1920 B contiguous, gpsimd SWDGE
    queue so output generation never queues behind input loads.
"""

import ml_dtypes
import numpy as np

import concourse.bacc as bacc
import concourse.mybir as mybir
import concourse.tile as tile
from concourse.bass_utils import run_bass_kernel_spmd

N = 64          # wires
OUT = 10        # measured wires / classes
NCORES = 8
JBLK = 48                  # matmul j-blocks per full super-block
TILE_W = JBLK * 128        # 6144 xstack cols per full super-block
# 20 full super-blocks + two 12-j tail blocks: minimal padding (0.76%)
# and a short serial drain at the end of the pipeline
WIDTHS = [JBLK] * 20 + [12, 12]
R = 128 * sum(WIDTHS)      # per-core rows = 125952
B_PAD = R * NCORES         # 1007616
F32 = mybir.dt.float32
BF16 = mybir.dt.bfloat16
NPBF16 = ml_dtypes.bfloat16


# ---------------------------------------------------------------- host math
def _bs_pass(n, start, int_params):
    i = np.arange(start, n - 1, 2)
    j = i + 1
    theta = int_params[3 * i]
    phi = int_params[3 * i + 1]
    ct, st = np.cos(theta), np.sin(theta)
    cp, sp = np.cos(phi), np.sin(phi)
    S = np.eye(2 * n)
    S[i, i] = ct
    S[i, j] = -cp * st
    S[i, n + j] = -sp * st
    S[j, i] = cp * st
    S[j, j] = ct
    S[j, n + i] = -sp * st
    S[n + i, j] = sp * st
    S[n + i, n + i] = ct
    S[n + i, n + j] = -cp * st
    S[n + j, i] = sp * st
    S[n + j, n + i] = cp * st
    S[n + j, n + j] = ct
    return S


def _layer_symplectic(n, int1, squeezes, int2):
    M = _bs_pass(n, 0, int1)
    M = _bs_pass(n, 1, int1) @ M
    c = np.concatenate([np.cos(int1[2::3]), np.ones(1)])
    s = np.concatenate([np.sin(int1[2::3]), np.zeros(1)])
    Rm = np.block([[np.diag(c), np.diag(-s)], [np.diag(s), np.diag(c)]])
    Sq = np.diag(np.concatenate([np.exp(-squeezes), np.exp(squeezes)]))
    M = Sq @ (Rm @ M)
    M = _bs_pass(n, 0, int2) @ M
    M = _bs_pass(n, 1, int2) @ M
    return M


def _affine_map(layers):
    n = N
    S = np.eye(2 * n)
    d = np.zeros(2 * n)
    for int1, sq, int2, disp in layers:
        M = _layer_symplectic(n, int1, sq, int2)
        S = M @ S
        d = M @ d
        d[:n] += 2.0 * disp
    return S, d


def _device_constants(layers):
    S, d = _affine_map(layers)
    w = np.arange(OUT)
    rows = np.concatenate([w, N + w])
    cov = S @ S.T
    cov_term = cov[w, w] + cov[N + w, N + w]            # (10,)
    W2 = S[rows, :N].T.astype(np.float32)               # (64, 20), msel' scale
    d20 = (d[rows] / 2.0).astype(np.float32)            # (20,)
    covc = (cov_term / 4.0 - 0.5).astype(np.float32)    # (10,)

    Wh = W2.astype(NPBF16)
    Wl = (W2 - Wh.astype(np.float32)).astype(NPBF16)
    wcat = np.zeros((128, 40), NPBF16)                  # [[Wh, Wl], [Wh, 0]]
    wcat[0:64, 0:20] = Wh
    wcat[0:64, 20:40] = Wl
    wcat[64:128, 0:20] = Wh

    dconst = np.ascontiguousarray(np.broadcast_to(
        np.tile(d20, JBLK), (128, 20 * JBLK))).astype(np.float32)
    cconst = np.ascontiguousarray(np.broadcast_to(
        np.tile(covc, JBLK), (128, 10 * JBLK))).astype(np.float32)
    return wcat, dconst, cconst


# ---------------------------------------------------------------- bass build
def build_nc(widths=None):
    widths = widths or WIDTHS
    rr = 128 * sum(widths)
    nc = bacc.Bacc("TRN2", target_bir_lowering=False)
    WC = 20 * JBLK                             # tw cols per super-block (960)
    OC = 10 * JBLK                             # out cols per super-block (480)
    xs = nc.dram_tensor("xs", (128, rr), BF16, kind="ExternalInput")
    wst = nc.dram_tensor("wcat", (128, 40), BF16, kind="ExternalInput")
    dcon = nc.dram_tensor("dconst", (128, WC), F32, kind="ExternalInput")
    ccon = nc.dram_tensor("covconst", (128, OC), F32, kind="ExternalInput")
    out = nc.dram_tensor("out", (128, (rr // 128) * 10), F32,
                         kind="ExternalOutput")

    Square = mybir.ActivationFunctionType.Square
    Relu = mybir.ActivationFunctionType.Relu
    Ln = mybir.ActivationFunctionType.Ln

    with tile.TileContext(nc) as tc:
        with (
            tc.tile_pool(name="const", bufs=1) as cpool,
            tc.tile_pool(name="xin", bufs=4) as xpool,
            tc.tile_pool(name="mid", bufs=3) as mpool,
            tc.tile_pool(name="ob", bufs=3) as opool,
            tc.tile_pool(name="ps", bufs=2, space="PSUM") as pspool,
        ):
            # w_t gates the first matmul: load it first on the sync queue
            # (the gpsimd queue can race its ucode load during the preamble)
            w_t = cpool.tile([128, 40], BF16)
            nc.sync.dma_start(w_t[:], wst[:])
            d_t = cpool.tile([128, WC], F32)
            nc.gpsimd.dma_start(d_t[:], dcon[:])
            c_t = cpool.tile([128, OC], F32)
            nc.gpsimd.dma_start(c_t[:], ccon[:])

            def emit_sb(col_base, jblk, in_chunks):
                wc, oc, nbank = 20 * jblk, 10 * jblk, jblk // 12
                w = 128 * jblk
                tin = xpool.tile([128, w], BF16, tag="tin")
                q = w // in_chunks
                for c4 in range(in_chunks):
                    nc.sync.dma_start(
                        tin[:, c4 * q:(c4 + 1) * q],
                        xs[:, col_base + c4 * q:col_base + (c4 + 1) * q])

                # psum: 12 j-blocks use the first 480 cols of each 512-col
                # bank (no bank crossing)
                ps = pspool.tile([128, nbank, 512], F32, tag="ps")
                for j in range(jblk):
                    nc.tensor.matmul(
                        ps[:, j // 12, 40 * (j % 12):40 * (j % 12) + 40],
                        tin[:, 128 * j:128 * j + 128], w_t[:],
                        start=True, stop=True,
                    )
                psv = ps[:, :, 0:480].rearrange(
                    "p t (g r k) -> p t g r k", r=2, k=20)
                dv = d_t[:, 0:wc].rearrange(
                    "p (t g k) -> p t g k", t=nbank, k=20)
                # t2 = r1 + (r2 + d): folds the d-add into the hi/lo merge
                t2 = mpool.tile([128, wc], F32, tag="t2")
                u = mpool.tile([128, wc], F32, tag="u")
                uv = u[:].rearrange("p (t g k) -> p t g k", t=nbank, k=20)
                nc.vector.tensor_add(uv, psv[:, :, :, 1, :], dv)
                t2v = t2[:].rearrange("p (t g k) -> p t g k", t=nbank, k=20)
                nc.vector.tensor_add(t2v, psv[:, :, :, 0, :], uv)

                sq = mpool.tile([128, wc], F32, tag="sq")
                nc.scalar.activation(sq[:], t2[:], Square)
                sqg = sq[:].rearrange("p (g k) -> p g k", k=20)
                s = mpool.tile([128, oc], F32, tag="s")
                sv = s[:].rearrange("p (g k) -> p g k", k=10)
                nc.vector.tensor_add(sv, sqg[:, :, 0:10], sqg[:, :, 10:20])
                v = mpool.tile([128, oc], F32, tag="v")
                nc.vector.tensor_add(v[:], s[:], c_t[:, 0:oc])
                r = mpool.tile([128, oc], F32, tag="r")
                nc.scalar.activation(r[:], v[:], Relu)
                o = opool.tile([128, oc], F32, tag="o")
                nc.scalar.activation(o[:], r[:], Ln, bias=1.0)

                ob = (col_base // 128) * 10
                nc.gpsimd.dma_start(out[:, ob:ob + oc], o[:])

            # first tile's DMA in eighths so compute starts sooner
            col = 0
            for i, wdt in enumerate(widths):
                emit_sb(col, wdt, 8 if i == 0 else 1)
                col += 128 * wdt
    nc.compile()
    return nc


# ---------------------------------------------------------------- host glue
def _make_in_maps(x_batch, wcat, dconst, cconst):
    B = x_batch.shape[0]
    xpad = np.zeros((B_PAD, N), np.float32)
    xpad[:B] = x_batch
    xh = xpad.astype(NPBF16)
    xl = (xpad - xh.astype(np.float32)).astype(NPBF16)
    in_maps = []
    for c in range(NCORES):
        sl = slice(c * R, (c + 1) * R)
        xstk = np.empty((128, R), NPBF16)
        xstk[0:64] = xh[sl].T
        xstk[64:128] = xl[sl].T
        in_maps.append({"xs": xstk, "wcat": wcat,
                        "dconst": dconst, "covconst": cconst})
    return in_maps


def _decode_out(results, B):
    full = np.empty((B_PAD, OUT), np.float32)
    for c in range(NCORES):
        O = results[c]["out"].reshape(128, R // 128, OUT)
        rows = O.transpose(1, 0, 2).reshape(R, OUT)
        full[c * R:(c + 1) * R] = rows
    return full[:B]


_NC_CACHE = {}


def kernel(x_batch, int1_0, squeezes_0, int2_0, disp_0,
           int1_1, squeezes_1, int2_1, disp_1, _trace=False):
    layers = [
        (np.asarray(int1_0, np.float64), np.asarray(squeezes_0, np.float64),
         np.asarray(int2_0, np.float64), np.asarray(disp_0, np.float64)),
        (np.asarray(int1_1, np.float64), np.asarray(squeezes_1, np.float64),
         np.asarray(int2_1, np.float64), np.asarray(disp_1, np.float64)),
    ]
    wcat, dconst, cconst = _device_constants(layers)
    in_maps = _make_in_maps(np.asarray(x_batch, np.float32), wcat, dconst, cconst)

    if "nc" not in _NC_CACHE:
        _NC_CACHE["nc"] = build_nc()
    nc = _NC_CACHE["nc"]

    res = run_bass_kernel_spmd(
        nc, in_maps, core_ids=list(range(NCORES)), trace=_trace
    )
    out = _decode_out(res.results, x_batch.shape[0])
    if _trace:
        return out, res
    return out

